# revision 4
# baseline (speedup 1.0000x reference)
"""Trainium2 Bass kernel for ContinuousAttentiveStatisticsPooling.

Shape config (hardcoded): B=8, C=256, L=8192, A=128, 8 NeuronCores.

Length-balanced sharding: lengths are ~U(0.5,1), so instead of one
example per core (every core paying full L), the valid column ranges of
ALL examples are cut into W=896-wide slots and dealt evenly across the
8 cores (56 slots -> 7 per core for the reference lengths). Partial
softmax statistics (Z, S1, S2) are linear in l, so the host merges the
per-slot partials per example and runs the (tiny) finalize math in
numpy - the device streams slots and dumps raw accumulators only.

Math restructure (per example, x is [C, L]):
  - Host zeroes x beyond the valid length -> all L-reductions over
    processed columns equal masked reductions plus an exactly-known
    pinv contribution from processed-but-invalid (zero) columns.
  - Host folds input-moment/weight-only terms:
      gmean = sum(x)/total ; gstd = sqrt(clip(sum(x^2)/total - gmean^2))
      ch   = Wt2 @ gmean + Wt3 @ gstd + b_tdnn          (relu bias)
      cv   = W2 @ gmean + W3 @ gstd + b_val             (values const)
      pinv = exp(Wc' @ relu(ch))                        (invalid-col p)
  - Device streams its slots once (slot k of example b):
      vraw   = W1 @ x                   (values, raw: cv added on host)
      h      = relu(Wt1 @ x + ch_b)
      p      = exp(Wc' @ h)             (score bias b' dropped: a
                                         per-channel constant cancels in
                                         the softmax over L)
      Z[k] += p ; S1[k] += p*vraw ; S2[k] += (p*vraw)*vraw
  - Host: Z_b = sum_slots Z - n_invalid_b * pinv_b ;
      amean = S1/Z + cv ; avar = S2/Z - (S1/Z)^2 ; astd = sqrt(avar)

Schedule notes (from HW traces):
  - Each issuing engine owns ONE in-order DMA ring (~350GB/s steady
    after a ramp); slot chunks split across sync/gpsimd rings, weights
    ahead of them, slot 0 on the otherwise-idle scalar ring.
  - PSUM = v(2 tiles x 2 banks) + ph/s(2 x 2) = 8 banks; [128,1024]
    fp32 tiles used 896 wide so matmul halves (512, 384) stay
    bank-aligned.
  - Matmuls ordered k-outer so each stationary weight loads once per
    slot (halves LDWEIGHTS count).
  - exp only (no Sqrt): Square/Exp/Relu share one ACT table set.
"""

import sys

if "/opt/trn_rl_repo" not in sys.path:
    sys.path.insert(0, "/opt/trn_rl_repo")

import numpy as np
import ml_dtypes

import concourse.bass as bass
import concourse.mybir as mybir
import concourse.tile as tile
from concourse.bass_utils import run_bass_kernel_spmd

B, C, L, A = 8, 256, 8192, 128
CB = C // 128          # 2 c-blocks
W = 896                # slot width (columns per streaming superblock)
H0, H1 = 512, 384      # matmul halves, bank-aligned in PSUM
NCORES = 8
EPS = 1e-12
F32 = mybir.dt.float32
BF16 = mybir.dt.bfloat16
ALU = mybir.AluOpType
ACT = mybir.ActivationFunctionType

_mw_ctr = [0]


def _split_multiwaits(nc):
    """This walrus build supports only ONE sync-wait per instruction.
    Split multi-wait instructions into single-wait NoOps on the same engine
    (same-engine program order preserves semantics exactly)."""
    for f in nc.m.functions:
        for blk in f.blocks:
            insts = blk.instructions
            out = []
            changed = False
            for inst in insts:
                si = inst.sync_info
                if si is not None and len(si.on_wait) > 1:
                    changed = True
                    waits = list(si.on_wait)
                    for w in waits[:-1]:
                        _mw_ctr[0] += 1
                        nop = mybir.InstNoOp(
                            name=f"mwsplit-{_mw_ctr[0]}", ins=[], outs=[]
                        )
                        nop.engine = inst.engine
                        nop.sync_info = mybir.SyncInfo(on_wait=[w], on_update=[])
                        out.append(nop)
                    inst.sync_info = mybir.SyncInfo(
                        on_wait=[waits[-1]], on_update=list(si.on_update)
                    )
                out.append(inst)
            if changed:
                insts[:] = out


def _build_nc(n_sb):
    nc = bass.Bass()
    # x slots: [128, n_sb, CB, W] so one slot is a contiguous
    # 2*W*2B = 3.5KB per-partition DMA chunk
    x_d = nc.dram_tensor("xp", [128, n_sb, CB, W], BF16, kind="ExternalInput")
    wv1t_d = nc.dram_tensor("wv1t", [128, 2, CB, 128], BF16, kind="ExternalInput")
    wtt_d = nc.dram_tensor("wtt", [128, 2, 128], BF16, kind="ExternalInput")
    wct_d = nc.dram_tensor("wct", [128, CB, 128], BF16, kind="ExternalInput")
    chs_d = nc.dram_tensor("chs", [128, n_sb], F32, kind="ExternalInput")
    # raw accumulators out: [(stat,cb) = 6, slot]
    out_d = nc.dram_tensor("out", [128, 6 * n_sb], F32, kind="ExternalOutput")

    with tile.TileContext(nc) as tc:
        with (
            tc.tile_pool(name="consts", bufs=1) as cp,
            tc.tile_pool(name="xs", bufs=1) as xp,
            tc.tile_pool(name="hw", bufs=3) as hp,
            tc.tile_pool(name="pw", bufs=4) as pp,
            tc.tile_pool(name="qw", bufs=4) as qp,
            tc.tile_pool(name="q2w", bufs=2) as q2p,
        ):
            # ---- DMAs across THREE in-order rings (sync / gpsimd /
            # scalar): slot 0 rides the otherwise-idle scalar ring so
            # streaming starts early; weights lead their ring. ----
            zz = cp.tile([128, 1], F32, tag="zz", name="zz")
            nc.vector.memset(zz, 0)
            zzo = cp.tile([128, 1], F32, tag="zzo", name="zzo")
            # dummy activation: forces the ACT table load at t~0
            nc.scalar.activation(out=zzo, in_=zz, func=ACT.Relu)

            # slot 0 + the weights it needs lead the two fast rings so
            # the first matmul chain can start ~2us in; later slots
            # alternate rings behind them.
            xs = [xp.tile([128, CB, W], BF16, tag=f"x_{k}", name=f"x_{k}")
                  for k in range(n_sb)]
            nc.sync.dma_start(out=xs[0], in_=x_d[:, 0, :, :])
            wtt = cp.tile([128, 2, 128], BF16, tag="wtt", name="wtt")
            nc.sync.dma_start(out=wtt, in_=wtt_d[:, :, :])
            chs = cp.tile([128, n_sb], F32, tag="chs", name="chs")
            nc.sync.dma_start(out=chs, in_=chs_d[:, :])
            wv1t = cp.tile([128, 2, CB, 128], BF16, tag="wv1t", name="wv1t")
            nc.gpsimd.dma_start(out=wv1t, in_=wv1t_d[:, :, :, :])
            wct = cp.tile([128, CB, 128], BF16, tag="wct", name="wct")
            nc.gpsimd.dma_start(out=wct, in_=wct_d[:, :, :])
            for k in range(1, n_sb):
                eng = nc.gpsimd if k % 2 == 1 else nc.sync
                eng.dma_start(out=xs[k], in_=x_d[:, k, :, :])

            # streaming accumulators (2D tiles: accum_out must be 2D)
            # layout matches out_d: [(stat,cb), slot]
            stat = cp.tile([128, 6 * n_sb], F32, tag="stat", name="stat")

            def acc(stat_i, cb, k):
                col = (stat_i * CB + cb) * n_sb + k
                return stat[:, col : col + 1]

            with (
                tc.tile_pool(name="psv", bufs=2, space="PSUM") as ps_v,
                tc.tile_pool(name="pss", bufs=2, space="PSUM") as ps_s,
            ):
                halves = [slice(0, H0), slice(H0, W)]

                def emit_ph(k):
                    # [128,1024] alloc keeps matmul outputs bank-aligned
                    ph = ps_s.tile([128, 1024], F32, tag="s", name="ph")
                    for kb in range(2):
                        for hsl in halves:
                            nc.tensor.matmul(ph[:, hsl], lhsT=wtt[:, kb, :],
                                             rhs=xs[k][:, kb, hsl],
                                             start=(kb == 0), stop=(kb == 1))
                    return ph

                def emit_v(k, cb):
                    vps = ps_v.tile([128, 1024], F32, tag="v", name="v")
                    for kb in range(2):
                        for hsl in halves:
                            nc.tensor.matmul(vps[:, hsl], lhsT=wv1t[:, kb, cb, :],
                                             rhs=xs[k][:, kb, hsl],
                                             start=(kb == 0), stop=(kb == 1))
                    return vps

                ph_next = emit_ph(0)
                v_next = {cb: emit_v(0, cb) for cb in range(CB)}

                for k in range(n_sb):
                    ph = ph_next
                    vk = v_next
                    h = hp.tile([128, W], BF16, tag="h", name="h")
                    nc.scalar.activation(out=h, in_=ph[:, 0:W], func=ACT.Relu,
                                         bias=chs[:, k : k + 1])
                    for cb in range(CB):
                        sps = ps_s.tile([128, 1024], F32, tag="s", name="s")
                        for hsl in halves:
                            nc.tensor.matmul(sps[:, hsl], lhsT=wct[:, cb, :],
                                             rhs=h[:, hsl], start=True, stop=True)
                        if cb == 0 and k + 1 < n_sb:
                            ph_next = emit_ph(k + 1)
                        p = pp.tile([128, W], BF16, tag="p", name="p")
                        nc.scalar.activation(
                            out=p, in_=sps[:, 0:W], func=ACT.Exp,
                            accum_out=acc(0, cb, k),
                        )
                        q = qp.tile([128, W], BF16, tag="q", name="q")
                        nc.vector.scalar_tensor_tensor(
                            out=q, in0=p, scalar=0.0, in1=vk[cb][:, 0:W],
                            op0=ALU.bypass, op1=ALU.mult,
                            accum_out=acc(1, cb, k),
                        )
                        q2 = q2p.tile([128, W], BF16, tag="q2", name="q2")
                        nc.vector.scalar_tensor_tensor(
                            out=q2, in0=q, scalar=0.0, in1=vk[cb][:, 0:W],
                            op0=ALU.bypass, op1=ALU.mult,
                            accum_out=acc(2, cb, k),
                        )
                    if k + 1 < n_sb:
                        v_next = {cb: emit_v(k + 1, cb) for cb in range(CB)}

            nc.scalar.dma_start(out=out_d[:, :], in_=stat)

    _split_multiwaits(nc)
    return nc


_NC_CACHE = {}


def _get_nc(n_sb):
    if n_sb not in _NC_CACHE:
        _NC_CACHE[n_sb] = _build_nc(n_sb)
    return _NC_CACHE[n_sb]


def _prep_inputs(x, lengths, w_val, b_val, w_tdnn, b_tdnn, bn_gamma, bn_beta,
                 w_conv, b_conv):
    x = np.asarray(x, dtype=np.float32)
    lengths = np.asarray(lengths, dtype=np.float32)
    w_val = np.asarray(w_val, dtype=np.float32)
    b_val = np.asarray(b_val, dtype=np.float32)
    w_tdnn = np.asarray(w_tdnn, dtype=np.float32)
    b_tdnn = np.asarray(b_tdnn, dtype=np.float32)
    bn_gamma = np.asarray(bn_gamma, dtype=np.float32)
    bn_beta = np.asarray(bn_beta, dtype=np.float32)
    w_conv = np.asarray(w_conv, dtype=np.float32)
    b_conv = np.asarray(b_conv, dtype=np.float32)

    mask = (np.arange(L, dtype=np.float32)[None, :] < (lengths * L)[:, None])
    total = mask.sum(axis=1).astype(np.int64)               # [B]
    xm = (x * mask[:, None, :].astype(np.float32)).astype(ml_dtypes.bfloat16)
    xf = xm.astype(np.float32)

    # masked global moments (from the bf16-rounded x the device also sees)
    totf = total.astype(np.float32)
    gmean = xf.sum(axis=2) / totf[:, None]                                   # [B, C]
    gsq = (xf * xf).sum(axis=2) / totf[:, None]
    gstd = np.sqrt(np.clip(gsq - gmean * gmean, EPS, None))                  # [B, C]

    def pack_lhsT(w, kblocks, cblocks, dt=None):
        # w: [K, M] (contraction-major) -> [128, kblocks, cblocks, 128]
        Ktot, Mtot = w.shape
        assert Ktot == kblocks * 128 and Mtot == cblocks * 128
        r = np.ascontiguousarray(
            w.reshape(kblocks, 128, cblocks, 128).transpose(1, 0, 2, 3)
        )
        return r.astype(dt) if dt is not None else r

    W1T = w_val[:, :C].T                                   # [f, c]
    wv1t = pack_lhsT(W1T, 2, CB, ml_dtypes.bfloat16)
    WtT = w_tdnn[:, :C].T                                  # [f, a]
    wtt = pack_lhsT(WtT, 2, 1, ml_dtypes.bfloat16).reshape(128, 2, 128)
    WcT = (w_conv * bn_gamma[None, :]).T                   # [a, c] (BN gamma folded)
    wct = pack_lhsT(WcT, 1, CB, ml_dtypes.bfloat16).reshape(128, CB, 128)
    # score bias b' = b_conv + w_conv @ bn_beta is constant per channel
    # -> cancels in the softmax; not needed anywhere.

    # per-example folded consts
    chs_b = np.empty((B, A), np.float32)
    cv_b = np.empty((B, C), np.float32)
    pinv_b = np.empty((B, C), np.float32)
    for b in range(B):
        gcat = np.concatenate([gmean[b], gstd[b]])                           # [2C]
        ch = w_tdnn[:, C:] @ gcat + b_tdnn                                   # [A]
        cv_b[b] = w_val[:, C:] @ gcat + b_val                                # [C]
        hinv = np.maximum(ch, 0.0).astype(ml_dtypes.bfloat16).astype(np.float32)
        pinv_b[b] = WcT.astype(ml_dtypes.bfloat16).astype(np.float32).T @ hinv
        chs_b[b] = ch
    pinv_b = np.exp(pinv_b)

    # ---- slot assignment: cut valid ranges into W-wide slots, deal
    # round-robin across cores ----
    slots = []                                              # (b, l0, width)
    for b in range(B):
        l0 = 0
        while l0 < total[b]:
            slots.append((b, l0, int(min(W, total[b] - l0))))
            l0 += W
    n_sb = (len(slots) + NCORES - 1) // NCORES

    shared = {"wv1t": wv1t, "wtt": wtt, "wct": wct}
    in_maps = []
    slot_map = []                                           # per core: [(b, width)]
    for core in range(NCORES):
        mine = slots[core::NCORES]
        xp = np.zeros((128, n_sb, CB, W), dtype=ml_dtypes.bfloat16)
        chs = np.zeros((128, n_sb), dtype=np.float32)
        smap = []
        for k, (b, l0, w) in enumerate(mine):
            sl = xm[b, :, l0 : l0 + w].reshape(CB, 128, w)
            xp[:, k, :, :w] = sl.transpose(1, 0, 2)
            chs[:, k] = chs_b[b]
            smap.append((b, w))
        m = dict(shared)
        m["xp"] = np.ascontiguousarray(xp)
        m["chs"] = np.ascontiguousarray(chs)
        in_maps.append(m)
        slot_map.append(smap)
    return in_maps, n_sb, slot_map, total, cv_b, pinv_b


def kernel(**inputs) -> np.ndarray:
    in_maps, n_sb, slot_map, total, cv_b, pinv_b = _prep_inputs(**inputs)
    nc = _get_nc(n_sb)
    res = run_bass_kernel_spmd(nc, in_maps, core_ids=list(range(NCORES)))
    # merge per-slot partials per example (host-side finalize)
    Z = np.zeros((B, C), np.float64)
    S1 = np.zeros((B, C), np.float64)
    S2 = np.zeros((B, C), np.float64)
    nproc = np.zeros(B, np.int64)
    for core in range(NCORES):
        o = res.results[core]["out"].astype(np.float64)     # [128, 6*n_sb]
        o = o.reshape(128, 3, CB, n_sb)
        for k, (b, w) in enumerate(slot_map[core]):
            # stat columns are [c-block major] -> channel = cb*128 + p
            Z[b] += o[:, 0, :, k].T.reshape(C)
            S1[b] += o[:, 1, :, k].T.reshape(C)
            S2[b] += o[:, 2, :, k].T.reshape(C)
            nproc[b] += W
    n_inv = (nproc - total).astype(np.float64)              # zero-padded cols
    Zv = Z - n_inv[:, None] * pinv_b.astype(np.float64)
    m1 = S1 / Zv
    amean = m1 + cv_b
    avar = np.clip(S2 / Zv - m1 * m1, EPS, None)
    astd = np.sqrt(avar)
    out = np.concatenate([amean, astd], axis=1).astype(np.float32)
    return out[:, :, None]


# revision 11
# speedup vs baseline: 1.0564x; 1.0564x over previous
"""Trainium2 Bass kernel for ContinuousAttentiveStatisticsPooling.

Shape config (hardcoded): B=8, C=256, L=8192, A=128, 8 NeuronCores.

Length-balanced sharding: lengths are ~U(0.5,1), so instead of one
example per core (every core paying full L), the valid column ranges of
ALL examples are cut into W=896-wide slots and dealt evenly across the
8 cores (56 slots -> 7 per core for the reference lengths). Partial
softmax statistics (Z, S1, S2) are linear in l, so the host merges the
per-slot partials per example and runs the (tiny) finalize math in
numpy - the device streams slots and dumps raw accumulators only.

Math restructure (per example, x is [C, L]):
  - Host zeroes x beyond the valid length -> all L-reductions over
    processed columns equal masked reductions plus an exactly-known
    pinv contribution from processed-but-invalid (zero) columns.
  - Host folds input-moment/weight-only terms:
      gmean = sum(x)/total ; gstd = sqrt(clip(sum(x^2)/total - gmean^2))
      ch   = Wt2 @ gmean + Wt3 @ gstd + b_tdnn          (relu bias)
      cv   = W2 @ gmean + W3 @ gstd + b_val             (values const)
      pinv = exp(Wc' @ relu(ch))                        (invalid-col p)
  - Device streams its slots once (slot k of example b):
      vraw   = W1 @ x                   (values, raw: cv added on host)
      h      = relu(Wt1 @ x + ch_b)
      p      = exp(Wc' @ h)             (score bias b' dropped: a
                                         per-channel constant cancels in
                                         the softmax over L)
      Z[k] += p ; S1[k] += p*vraw ; S2[k] += (p*vraw)*vraw
  - Host: Z_b = sum_slots Z - n_invalid_b * pinv_b ;
      amean = S1/Z + cv ; avar = S2/Z - (S1/Z)^2 ; astd = sqrt(avar)

Schedule notes (from HW traces):
  - Each issuing engine owns ONE in-order DMA ring (~350GB/s steady
    after a ramp); slot chunks split across sync/gpsimd rings, weights
    ahead of them, slot 0 on the otherwise-idle scalar ring.
  - PSUM = v(2 tiles x 2 banks) + ph/s(2 x 2) = 8 banks; [128,1024]
    fp32 tiles used 896 wide so matmul halves (512, 384) stay
    bank-aligned.
  - Matmuls ordered k-outer so each stationary weight loads once per
    slot (halves LDWEIGHTS count).
  - exp only (no Sqrt): Square/Exp/Relu share one ACT table set.
"""

import sys

if "/opt/trn_rl_repo" not in sys.path:
    sys.path.insert(0, "/opt/trn_rl_repo")

import numpy as np
import ml_dtypes

import concourse.bass as bass
import concourse.mybir as mybir
import concourse.tile as tile
from concourse.bass_utils import run_bass_kernel_spmd

B, C, L, A = 8, 256, 8192, 128
CB = C // 128          # 2 c-blocks
W = 896                # slot width (columns per streaming superblock)
H0, H1 = 512, 384      # matmul halves, bank-aligned in PSUM
NCORES = 8
EPS = 1e-12
F32 = mybir.dt.float32
BF16 = mybir.dt.bfloat16
FP8 = mybir.dt.float8e4
NP_FP8 = ml_dtypes.float8_e4m3
DR = mybir.MatmulPerfMode.DoubleRow
ALU = mybir.AluOpType
ACT = mybir.ActivationFunctionType

_mw_ctr = [0]


def _split_multiwaits(nc):
    """This walrus build supports only ONE sync-wait per instruction.
    Split multi-wait instructions into single-wait NoOps on the same engine
    (same-engine program order preserves semantics exactly)."""
    for f in nc.m.functions:
        for blk in f.blocks:
            insts = blk.instructions
            out = []
            changed = False
            for inst in insts:
                si = inst.sync_info
                if si is not None and len(si.on_wait) > 1:
                    changed = True
                    waits = list(si.on_wait)
                    for w in waits[:-1]:
                        _mw_ctr[0] += 1
                        nop = mybir.InstNoOp(
                            name=f"mwsplit-{_mw_ctr[0]}", ins=[], outs=[]
                        )
                        nop.engine = inst.engine
                        nop.sync_info = mybir.SyncInfo(on_wait=[w], on_update=[])
                        out.append(nop)
                    inst.sync_info = mybir.SyncInfo(
                        on_wait=[waits[-1]], on_update=list(si.on_update)
                    )
                out.append(inst)
            if changed:
                insts[:] = out


def _build_nc(n_sb):
    nc = bass.Bass()
    # x slots: [128, n_sb, CB, W] fp8 so one slot is a contiguous
    # 2*W = 1.75KB per-partition DMA chunk; the x-side matmuls run in
    # fp8 DoubleRow mode (whole K=256 contraction in one matmul)
    x_d = nc.dram_tensor("xp", [128, n_sb, CB, W], FP8, kind="ExternalInput")
    wv1t_d = nc.dram_tensor("wv1t", [128, 2, CB, 128], FP8, kind="ExternalInput")
    wtt_d = nc.dram_tensor("wtt", [128, 2, 128], FP8, kind="ExternalInput")
    wct_d = nc.dram_tensor("wct", [128, CB, 128], BF16, kind="ExternalInput")
    chs_d = nc.dram_tensor("chs", [128, n_sb], F32, kind="ExternalInput")
    # raw accumulators out: [(stat,cb) = 6, slot]
    out_d = nc.dram_tensor("out", [128, 6 * n_sb], F32, kind="ExternalOutput")

    with tile.TileContext(nc) as tc:
        with (
            tc.tile_pool(name="consts", bufs=1) as cp,
            tc.tile_pool(name="xs", bufs=1) as xp,
            tc.tile_pool(name="hw", bufs=3) as hp,
            tc.tile_pool(name="pw", bufs=4) as pp,
            tc.tile_pool(name="qw", bufs=4) as qp,
            tc.tile_pool(name="q2w", bufs=2) as q2p,
        ):
            # ---- DMAs across THREE in-order rings (sync / gpsimd /
            # scalar): slot 0 rides the otherwise-idle scalar ring so
            # streaming starts early; weights lead their ring. ----
            zz = cp.tile([128, 1], F32, tag="zz", name="zz")
            nc.vector.memset(zz, 0)
            zzo = cp.tile([128, 1], F32, tag="zzo", name="zzo")
            # dummy activation: forces the ACT table load at t~0
            nc.scalar.activation(out=zzo, in_=zz, func=ACT.Relu)

            # slot 0 (split by matmul halves across both fast rings) +
            # the weights it needs lead the rings so the first matmul
            # chain starts ~2us in; later slots alternate rings.
            xs = [xp.tile([128, CB, W], FP8, tag=f"x_{k}", name=f"x_{k}")
                  for k in range(n_sb)]
            nc.sync.dma_start(out=xs[0][:, :, 0:H0], in_=x_d[:, 0, :, 0:H0])
            wtt = cp.tile([128, 2, 128], FP8, tag="wtt", name="wtt")
            nc.sync.dma_start(out=wtt, in_=wtt_d[:, :, :])
            chs = cp.tile([128, n_sb], F32, tag="chs", name="chs")
            nc.sync.dma_start(out=chs, in_=chs_d[:, :])
            nc.gpsimd.dma_start(out=xs[0][:, :, H0:W], in_=x_d[:, 0, :, H0:W])
            wv1t = cp.tile([128, 2, CB, 128], FP8, tag="wv1t", name="wv1t")
            nc.gpsimd.dma_start(out=wv1t, in_=wv1t_d[:, :, :, :])
            wct = cp.tile([128, CB, 128], BF16, tag="wct", name="wct")
            nc.gpsimd.dma_start(out=wct, in_=wct_d[:, :, :])
            for k in range(1, n_sb):
                eng = nc.gpsimd if k % 2 == 1 else nc.sync
                eng.dma_start(out=xs[k], in_=x_d[:, k, :, :])

            # streaming accumulators (2D tiles: accum_out must be 2D)
            # layout matches out_d: [(stat,cb), slot]
            stat = cp.tile([128, 6 * n_sb], F32, tag="stat", name="stat")

            def acc(stat_i, cb, k):
                col = (stat_i * CB + cb) * n_sb + k
                return stat[:, col : col + 1]

            with (
                tc.tile_pool(name="psv", bufs=2, space="PSUM") as ps_v,
                tc.tile_pool(name="pss", bufs=2, space="PSUM") as ps_s,
            ):
                halves = [slice(0, H0), slice(H0, W)]

                def emit_ph(k):
                    # [128,1024] alloc keeps matmul outputs bank-aligned;
                    # DoubleRow: one matmul covers both 128-kblocks
                    ph = ps_s.tile([128, 1024], F32, tag="s", name="ph")
                    for hsl in halves:
                        nc.tensor.matmul(ph[:, hsl], lhsT=wtt[:, :, :],
                                         rhs=xs[k][:, :, hsl],
                                         start=True, stop=True, perf_mode=DR)
                    return ph

                def emit_v(k, cb):
                    vps = ps_v.tile([128, 1024], F32, tag="v", name="v")
                    for hsl in halves:
                        nc.tensor.matmul(vps[:, hsl], lhsT=wv1t[:, :, cb, :],
                                         rhs=xs[k][:, :, hsl],
                                         start=True, stop=True, perf_mode=DR)
                    return vps

                ph_next = emit_ph(0)
                v_next = {cb: emit_v(0, cb) for cb in range(CB)}

                for k in range(n_sb):
                    ph = ph_next
                    vk = v_next
                    h = hp.tile([128, W], BF16, tag="h", name="h")
                    nc.scalar.activation(out=h, in_=ph[:, 0:W], func=ACT.Relu,
                                         bias=chs[:, k : k + 1])
                    for cb in range(CB):
                        sps = ps_s.tile([128, 1024], F32, tag="s", name="s")
                        for hsl in halves:
                            nc.tensor.matmul(sps[:, hsl], lhsT=wct[:, cb, :],
                                             rhs=h[:, hsl], start=True, stop=True)
                        if cb == 0 and k + 1 < n_sb:
                            ph_next = emit_ph(k + 1)
                        p = pp.tile([128, W], BF16, tag="p", name="p")
                        nc.scalar.activation(
                            out=p, in_=sps[:, 0:W], func=ACT.Exp,
                            accum_out=acc(0, cb, k),
                        )
                        q = qp.tile([128, W], BF16, tag="q", name="q")
                        nc.vector.scalar_tensor_tensor(
                            out=q, in0=p, scalar=0.0, in1=vk[cb][:, 0:W],
                            op0=ALU.bypass, op1=ALU.mult,
                            accum_out=acc(1, cb, k),
                        )
                        q2 = q2p.tile([128, W], BF16, tag="q2", name="q2")
                        nc.vector.scalar_tensor_tensor(
                            out=q2, in0=q, scalar=0.0, in1=vk[cb][:, 0:W],
                            op0=ALU.bypass, op1=ALU.mult,
                            accum_out=acc(2, cb, k),
                        )
                    if k + 1 < n_sb:
                        v_next = {cb: emit_v(k + 1, cb) for cb in range(CB)}

            nc.scalar.dma_start(out=out_d[:, :], in_=stat)

    _split_multiwaits(nc)
    return nc


_NC_CACHE = {}


def _get_nc(n_sb):
    if n_sb not in _NC_CACHE:
        _NC_CACHE[n_sb] = _build_nc(n_sb)
    return _NC_CACHE[n_sb]


def _prep_inputs(x, lengths, w_val, b_val, w_tdnn, b_tdnn, bn_gamma, bn_beta,
                 w_conv, b_conv):
    x = np.asarray(x, dtype=np.float32)
    lengths = np.asarray(lengths, dtype=np.float32)
    w_val = np.asarray(w_val, dtype=np.float32)
    b_val = np.asarray(b_val, dtype=np.float32)
    w_tdnn = np.asarray(w_tdnn, dtype=np.float32)
    b_tdnn = np.asarray(b_tdnn, dtype=np.float32)
    bn_gamma = np.asarray(bn_gamma, dtype=np.float32)
    bn_beta = np.asarray(bn_beta, dtype=np.float32)
    w_conv = np.asarray(w_conv, dtype=np.float32)
    b_conv = np.asarray(b_conv, dtype=np.float32)

    mask = (np.arange(L, dtype=np.float32)[None, :] < (lengths * L)[:, None])
    total = mask.sum(axis=1).astype(np.int64)               # [B]
    xmf = x * mask[:, None, :].astype(np.float32)
    xm = xmf.astype(NP_FP8)                                 # device copy

    # masked global moments (exact x; only the host uses these)
    totf = total.astype(np.float32)
    gmean = xmf.sum(axis=2) / totf[:, None]                                  # [B, C]
    gsq = (xmf * xmf).sum(axis=2) / totf[:, None]
    gstd = np.sqrt(np.clip(gsq - gmean * gmean, EPS, None))                  # [B, C]

    def pack_lhsT(w, kblocks, cblocks, dt=None):
        # w: [K, M] (contraction-major) -> [128, kblocks, cblocks, 128]
        Ktot, Mtot = w.shape
        assert Ktot == kblocks * 128 and Mtot == cblocks * 128
        r = np.ascontiguousarray(
            w.reshape(kblocks, 128, cblocks, 128).transpose(1, 0, 2, 3)
        )
        return r.astype(dt) if dt is not None else r

    W1T = w_val[:, :C].T                                   # [f, c]
    wv1t = pack_lhsT(W1T, 2, CB, NP_FP8)
    WtT = w_tdnn[:, :C].T                                  # [f, a]
    wtt = pack_lhsT(WtT, 2, 1, NP_FP8).reshape(128, 2, 128)
    WcT = (w_conv * bn_gamma[None, :]).T                   # [a, c] (BN gamma folded)
    wct = pack_lhsT(WcT, 1, CB, ml_dtypes.bfloat16).reshape(128, CB, 128)
    # score bias b' = b_conv + w_conv @ bn_beta is constant per channel
    # -> cancels in the softmax; not needed anywhere.

    # per-example folded consts
    chs_b = np.empty((B, A), np.float32)
    cv_b = np.empty((B, C), np.float32)
    pinv_b = np.empty((B, C), np.float32)
    for b in range(B):
        gcat = np.concatenate([gmean[b], gstd[b]])                           # [2C]
        ch = w_tdnn[:, C:] @ gcat + b_tdnn                                   # [A]
        cv_b[b] = w_val[:, C:] @ gcat + b_val                                # [C]
        hinv = np.maximum(ch, 0.0).astype(ml_dtypes.bfloat16).astype(np.float32)
        pinv_b[b] = WcT.astype(ml_dtypes.bfloat16).astype(np.float32).T @ hinv
        chs_b[b] = ch
    pinv_b = np.exp(pinv_b)

    # ---- slot assignment: cut valid ranges into W-wide slots, deal
    # round-robin across cores ----
    slots = []                                              # (b, l0, width)
    for b in range(B):
        l0 = 0
        while l0 < total[b]:
            slots.append((b, l0, int(min(W, total[b] - l0))))
            l0 += W
    n_sb = (len(slots) + NCORES - 1) // NCORES

    shared = {"wv1t": wv1t, "wtt": wtt, "wct": wct}
    in_maps = []
    slot_map = []                                           # per core: [(b, width)]
    for core in range(NCORES):
        mine = slots[core::NCORES]
        xp = np.zeros((128, n_sb, CB, W), dtype=NP_FP8)
        chs = np.zeros((128, n_sb), dtype=np.float32)
        smap = []
        for k, (b, l0, w) in enumerate(mine):
            sl = xm[b, :, l0 : l0 + w].reshape(CB, 128, w)
            xp[:, k, :, :w] = sl.transpose(1, 0, 2)
            chs[:, k] = chs_b[b]
            smap.append((b, w))
        m = dict(shared)
        m["xp"] = np.ascontiguousarray(xp)
        m["chs"] = np.ascontiguousarray(chs)
        in_maps.append(m)
        slot_map.append(smap)
    return in_maps, n_sb, slot_map, total, cv_b, pinv_b


def kernel(**inputs) -> np.ndarray:
    in_maps, n_sb, slot_map, total, cv_b, pinv_b = _prep_inputs(**inputs)
    nc = _get_nc(n_sb)
    res = run_bass_kernel_spmd(nc, in_maps, core_ids=list(range(NCORES)))
    # merge per-slot partials per example (host-side finalize)
    Z = np.zeros((B, C), np.float64)
    S1 = np.zeros((B, C), np.float64)
    S2 = np.zeros((B, C), np.float64)
    nproc = np.zeros(B, np.int64)
    for core in range(NCORES):
        o = res.results[core]["out"].astype(np.float64)     # [128, 6*n_sb]
        o = o.reshape(128, 3, CB, n_sb)
        for k, (b, w) in enumerate(slot_map[core]):
            # stat columns are [c-block major] -> channel = cb*128 + p
            Z[b] += o[:, 0, :, k].T.reshape(C)
            S1[b] += o[:, 1, :, k].T.reshape(C)
            S2[b] += o[:, 2, :, k].T.reshape(C)
            nproc[b] += W
    n_inv = (nproc - total).astype(np.float64)              # zero-padded cols
    Zv = Z - n_inv[:, None] * pinv_b.astype(np.float64)
    m1 = S1 / Zv
    amean = m1 + cv_b
    avar = np.clip(S2 / Zv - m1 * m1, EPS, None)
    astd = np.sqrt(avar)
    out = np.concatenate([amean, astd], axis=1).astype(np.float32)
    return out[:, :, None]
